# revision 1
# baseline (speedup 1.0000x reference)
"""ConvTranspose2d (16,256,32,32) -> (16,128,66,66), stride 2, 4x4 kernel.

Strategy: data-parallel over batch, 2 images per core on 8 NeuronCores.

Math: y[b,co,2m+p,2n+q] = bias[co]
        + sum_{i,j in {0,1}} sum_ci x[b,ci,m-i,n-j] * w[ci,co,p+2i,q+2j]
for parity class (p,q) in {0,1}^2, m,n in [0,33).

Per image and parity class: output subgrid [128co x 33 x 33] computed as
3 row-chunks of 11 rows; each chunk is one PSUM accumulation group of
8 matmuls (2 ci-chunks x 4 taps (i,j)), K=128, M=128, N=11*33=363,
in float32r (full-rate fp32 on the PE at N>=256).  Shifted taps read a
zero-padded 34x34 SBUF copy of x (padded host-side) through a strided
2D access pattern, so no junk columns are computed.  PSUM->SBUF drain
is a DVE tensor_scalar_add fusing the bias add and the parity
de-interleave.

Overlap choices: weights are DMA'd per parity class in consumption
order; image 0 runs class-major and its output leaves as one DMA that
overlaps image 1's compute; image 1 runs row-band-major and its output
leaves as three 22-row band DMAs so only the last ~0.7MB trails the
final matmul.
"""

import numpy as np

import concourse.bass as bass
import concourse.bacc as bacc
import concourse.tile as tile
from concourse import mybir
from concourse.bass_utils import run_bass_kernel_spmd

N_CORES = 8
B_PER = 2  # images per core

F32 = mybir.dt.float32
F32R = mybir.dt.float32r

PW = 34            # padded x width (32 + 1 left + 1 right)
XLEN = PW * PW     # 1156 padded x elems per partition
XPAD = 1160        # sbuf/dram x free size (AP slack for the last chunk)
R = 11             # output parity rows per PSUM chunk
NCH = 3            # chunks: 3 * 11 = 33 parity rows
NF = R * PW        # 374 matmul free dim (fp32r needs a contiguous rhs,
                   # so the pad column rides along and is dropped on drain)


def _emit_group(nc, ps, wt, xp, p, q, r):
    """One PSUM accumulation group: 8 matmuls for class (p,q), chunk r."""
    m0 = R * r
    k = 0
    for c in range(2):
        for i in range(2):
            for j in range(2):
                off = (m0 - i + 1) * PW + (1 - j)
                nc.tensor.matmul(
                    ps[:],
                    wt[c][:, p, q, i, j, :],
                    xp[c][:, off:off + NF],
                    start=(k == 0),
                    stop=(k == 7),
                )
                k += 1


def _emit_class(nc, pss, wt, xp, p, q, rs):
    """Chunks `rs` of class (p,q), tap-major: consecutive matmuls share
    the stationary weights, so their LDWEIGHTS overlap in-flight matmuls
    instead of gating them.  pss[r] is the PSUM tile for chunk r."""
    k = 0
    for c in range(2):
        for i in range(2):
            for j in range(2):
                for r in rs:
                    off = (R * r - i + 1) * PW + (1 - j)
                    nc.tensor.matmul(
                        pss[r][:],
                        wt[c][:, p, q, i, j, :],
                        xp[c][:, off:off + NF],
                        start=(k == 0),
                        stop=(k == 7),
                        skip_group_check=True,
                    )
                k += 1


def build_nc(debug: bool = False) -> bass.Bass:
    nc = bacc.Bacc("TRN2", target_bir_lowering=False, debug=debug,
                   num_devices=N_CORES)

    # x arrives host-padded: 34x34 zero-border layout + tail pad, flat
    x_d = nc.declare_dram_parameter("x", [B_PER, 256, XPAD], F32R,
                                    isOutput=False)
    # w layout: [ci_chunk, ci, p, q, i, j, co]  (class-major taps)
    w_d = nc.declare_dram_parameter("w", [2, 128, 2, 2, 2, 2, 128], F32R,
                                    isOutput=False)
    b_d = nc.declare_dram_parameter("b", [128, 1], F32, isOutput=False)
    y_d = nc.declare_dram_parameter("y", [B_PER, 128, 66, 66], F32,
                                    isOutput=True)

    with tile.TileContext(nc) as tc:
        with (
            tc.tile_pool(name="wp", bufs=2) as wpool,
            tc.tile_pool(name="bp", bufs=1) as bpool,
            tc.tile_pool(name="xp", bufs=2 * B_PER) as xpool,
            tc.tile_pool(name="yp", bufs=1) as ypool,
            tc.tile_pool(name="ybp", bufs=NCH) as bandpool,
            tc.tile_pool(name="ps", bufs=7, space="PSUM") as ppool,
            tc.tile_pool(name="pw", bufs=1, space="PSUM") as warmpool,
        ):
            # PE warm-up: HAM starts the PE at 1.2GHz and only unthrottles
            # after ~3.4us of sustained activity.  Burn that window on dummy
            # bf16 matmuls during the input-DMA ramp so the real matmuls
            # start at 2.4GHz.
            wub = bpool.tile([128, 512], mybir.dt.bfloat16)
            nc.vector.memset(wub[:], 0.0)
            wps = warmpool.tile([128, 512], F32)
            for _ in range(8):
                nc.tensor.matmul(wps[:], wub[:, 0:128], wub[:],
                                 start=True, stop=True)
            # weight tiles; DMA'd per class in consumption order
            wt = [wpool.tile([128, 2, 2, 2, 2, 128], F32R, name=f"wt{c}", tag="wt")
                  for c in range(2)]

            # ~620ns of sequencer time per dma_start: round-robin the
            # input-DMA issues over four engines so they don't serialize
            issue_engines = [nc.sync, nc.scalar, nc.gpsimd]
            issue_i = [0]

            def dma_in(out, in_):
                eng = issue_engines[issue_i[0] % len(issue_engines)]
                issue_i[0] += 1
                eng.dma_start(out=out, in_=in_)

            def dma_w_class(p, q, per_tap: bool = False):
                for c in range(2):
                    if per_tap:
                        # first-consumed class: land the first 64KB tap
                        # fast so the first real matmul isn't gated on
                        # the whole 512KB class
                        for i in range(2):
                            for j in range(2):
                                dma_in(wt[c][:, p, q, i, j],
                                       w_d[c, :, p, q, i, j])
                    else:
                        dma_in(wt[c][:, p, q], w_d[c, :, p, q])

            xp = {}

            def dma_x(img, banded: bool):
                xp[img] = [
                    xpool.tile([128, XPAD], F32R, name=f"x{img}c{c}",
                               tag="xt")
                    for c in range(2)
                ]
                # row bands [0:13), [13:24), [24:34): chunk r only needs
                # padded rows up to r*R + 12, so compute can start after
                # the first band lands
                bands = [(0, 13 * PW), (13 * PW, 24 * PW), (24 * PW, XPAD)]
                for lo, hi in (bands if banded else [(0, XPAD)]):
                    for c in range(2):
                        dma_in(xp[img][c][:, lo:hi],
                               x_d[img, c * 128:(c + 1) * 128, lo:hi])

            # issue order = consumption order
            dma_w_class(0, 0)
            dma_x(0, banded=True)
            dma_w_class(0, 1)
            dma_w_class(1, 0)
            dma_w_class(1, 1)
            dma_x(1, banded=False)
            bt = bpool.tile([128, 1], F32)
            nc.sync.dma_start(out=bt[:], in_=b_d[:])

            def drain(ps, out_view):
                nc.vector.tensor_scalar_add(
                    out_view,
                    ps[:].rearrange("p (m n) -> p m n", n=PW)[:, :, 0:33],
                    bt[:],
                )

            # ---- image 0: class-major; single output DMA ----
            yt = ypool.tile([128, 66, 66], F32)
            for p in range(2):
                for q in range(2):
                    for r in range(NCH):
                        ps = ppool.tile([128, NF], F32)
                        _emit_group(nc, ps, wt, xp[0], p, q, r)
                        drain(ps, yt[:, p::2, q::2][:, R * r:R * (r + 1), :])
            nc.gpsimd.dma_start(out=y_d[0], in_=yt[:])

            # ---- image 1: band-major; banded output DMAs ----
            for r in range(NCH):
                band = bandpool.tile([128, 2 * R, 66], F32)
                for p in range(2):
                    for q in range(2):
                        ps = ppool.tile([128, NF], F32)
                        _emit_group(nc, ps, wt, xp[1], p, q, r)
                        drain(ps, band[:, p::2, q::2])
                nc.gpsimd.dma_start(
                    out=y_d[1][:, 2 * R * r:2 * R * (r + 1), :],
                    in_=band[:],
                )

    nc.compile()
    return nc


_nc_cache = None


def _get_nc():
    global _nc_cache
    if _nc_cache is None:
        _nc_cache = build_nc()
    return _nc_cache


def make_in_maps(x: np.ndarray, weight: np.ndarray, bias: np.ndarray):
    # w[ci,co,kh,kw] -> [c, ci', p, q, i, j, co]
    w6 = (
        weight.astype(np.float32, copy=False)
        .reshape(2, 128, 128, 2, 2, 2, 2)      # [c, ci', co, i, p, j, q]
        .transpose(0, 1, 4, 6, 3, 5, 2)        # -> [c, ci', p, q, i, j, co]
    )
    w_host = np.ascontiguousarray(w6)
    b_host = np.ascontiguousarray(
        bias.astype(np.float32, copy=False).reshape(128, 1)
    )
    x = np.asarray(x, dtype=np.float32)
    # host-side zero-pad into the 34x34(+tail) layout the kernel reads
    xpad = np.zeros((16, 256, XPAD), dtype=np.float32)
    xpad[:, :, :XLEN].reshape(16, 256, PW, PW)[:, :, 1:33, 1:33] = x
    return [
        {
            "x": np.ascontiguousarray(xpad[B_PER * i:B_PER * (i + 1)]),
            "w": w_host,
            "b": b_host,
        }
        for i in range(N_CORES)
    ]


def kernel(x: np.ndarray, weight: np.ndarray, bias: np.ndarray) -> np.ndarray:
    nc = _get_nc()
    in_maps = make_in_maps(x, weight, bias)
    res = run_bass_kernel_spmd(nc, in_maps, list(range(N_CORES)))
    out = np.concatenate([r["y"] for r in res.results], axis=0)
    return np.ascontiguousarray(out.astype(np.float32, copy=False))

